# revision 1
# baseline (speedup 1.0000x reference)
"""DRN layer kernel for 8 TRN2 NeuronCores.

Math (reference):
    T[j,k,l,m]   = exp(-w[j,k] * (s0[m]-s1[l])^2)
    Pw[i,j,k,l]  = sum_m T[j,k,l,m] * P[i,k,m]
    logsum[i,j,l]= sum_k log(Pw[i,j,k,l])
    out          = softmax_l(logsum + exponent_B[j,l])

Sharding: tensor-parallel over n_upper (j): 8 cores x 8 upper nodes each,
every core sees the full batch. T depends only on the weights, so it is
precomputed on the host and shipped per-core in matmul-ready layout.

Device math uses a bf16 matmul with an exactness trick: T = 1 + t with
|t| <= 0.11, so Pw = S + sum_m t*P where S = sum_m P. t and P are sent
as bf16 (quantization error scales with |t|, not |T|~1), and S rides in
the matmul as two extra contraction rows (hi/lo bf16 split) against
columns of ones, so PSUM receives near-fp32-accurate Pw at bf16 speed.

PSUM is drained by two routes: "A" sgs are logged directly on ScalarE;
"B" sgs feed VectorE running-product chains (log(prod) == sum of logs,
so one chunk-log covers ~9 k's). The k-sum accumulates over four
independent chains (GpSimd adds, two DMA inline-accumulate chains via
SWDGE CCE, VectorE adds) merged at the end; the batched softmax over l
uses an l=0 shift instead of a max-reduce and stride-0 broadcast APs.
"""

import numpy as np

B, NU, NL, QU, QL = 256, 64, 64, 64, 64
NCORES = 8
JLOC = NU // NCORES  # 8 upper nodes per core
JL = JLOC * QU       # 512 = packed (j, l) free dim
KDIM = QL + 2        # 66 = contraction: 64 m-rows + S_hi + S_lo rows
PW = B + JL          # 768 packed free width of PTT

NPAIR = NL // 2      # 32 k-pairs
NGRP = 8             # pair groups per ih; each group = 4 pairs = 8 k


def _build_program():
    import concourse.bass as bass
    import concourse.bacc as bacc
    import concourse.mybir as mybir
    from concourse.tile import TileContext

    f32 = mybir.dt.float32
    bf16 = mybir.dt.bfloat16
    AF = mybir.ActivationFunctionType

    nc = bacc.Bacc(None, target_bir_lowering=False)
    # two k's packed per DMA row-block: [32, 66, 1536] — each dma_start
    # costs ~770ns of serial issue time on the Sync engine, so fewer,
    # bigger DMAs; 4-k blocks measured worse (arrival latency stalls PE).
    PTT = nc.declare_dram_parameter("PTT", [NL // 2, KDIM, 2 * PW], bf16,
                                    isOutput=False)
    EB = nc.declare_dram_parameter("EB", [128, 2 * JL], f32, isOutput=False)
    OUT = nc.declare_dram_parameter("out", [2, 128, JL], f32, isOutput=True)

    # 32 super-groups (sg) of 2 k's each; PSUM tile [128, 2048] holds the 4
    # Pw quarters [ih0k0 | ih1k0 | ih0k1 | ih1k1]. Route A sgs are drained
    # by ScalarE (Ln straight from PSUM) + GpSimd/VectorE adds; route B sgs
    # feed VectorE running-product chains (one PSUM operand per op), whose
    # chunk-logs land on ScalarE every ~10 sgs.
    NSG = NL // 2
    # 14 route-A sgs; the last two sgs are A so the kernel tail is short
    # (a chunk-log landing on the final sg would serialize at the end).
    A_SET = {(i * 30) // 12 for i in range(12)} | {NSG - 2, NSG - 1}
    b_list = [sg for sg in range(NSG) if sg not in A_SET]
    CHUNK = (len(b_list) + 1) // 2  # 2 chunks of product depth <= 9
    b_chunk = {sg: i // CHUNK for i, sg in enumerate(b_list)}
    b_pos = {sg: i % CHUNK for i, sg in enumerate(b_list)}
    b_last = {sg: (i % CHUNK == CHUNK - 1) or (i == len(b_list) - 1)
              for i, sg in enumerate(b_list)}

    with TileContext(nc) as tc:
        with (
            tc.tile_pool(name="ptt", bufs=8) as ppool,
            tc.tile_pool(name="eb", bufs=1) as ebpool,
            tc.tile_pool(name="ps", bufs=4, space="PSUM") as pspool,
            tc.tile_pool(name="lg", bufs=12) as lgpool,
            tc.tile_pool(name="clg", bufs=2) as clgpool,
            tc.tile_pool(name="pb", bufs=1) as pbpool,
            tc.tile_pool(name="acc", bufs=1) as apool,
            tc.tile_pool(name="sm", bufs=4) as smpool,
            tc.tile_pool(name="ot", bufs=4) as opool,
        ):
            # acc_g / acc_d are [128, 1024] = [ih0 | ih1]; a log tile's half
            # [:, h*1024:(h+1)*1024] = [ih0 k_h | ih1 k_h] lines up with
            # them. Two accumulators so the GpSimd and VectorE add chains
            # run independently; merged once at the end.
            ebt = ebpool.tile([128, 2 * JL], f32, tag="ebt")
            nc.sync.dma_start(out=ebt[:], in_=EB[:, :])
            acc_g = apool.tile([128, 2 * JL], f32, tag="accg", name="accg")
            acc_d = apool.tile([128, 2 * JL], f32, tag="accd", name="accd")
            acc_m = [apool.tile([128, 2 * JL], f32, tag=f"accm{i}",
                                name=f"accm{i}") for i in range(2)]
            for am in acc_m:
                nc.vector.memset(am[:], 0.0)
            acc_state = {"g": False, "d": False}
            add_rr = 0
            # Add-halves routed over four independent accumulator chains:
            # DMA inline-accumulate (SWDGE CCE add; GpSimd only preps the
            # descriptors, ~1us, cheaper than its own 2.7us add), GpSimd
            # direct, and VectorE. The final adds go to VectorE so the
            # slow GpSimd/DMA chains are off the kernel tail.
            # steady-state mix, then a GpSimd-free tail: GpSimd's last
            # tensor_tensor carries a ~7us pipeline drain, so its chain
            # must finish well before the kernel tail.
            ROUTE = ("m0", "g", "m1", "d", "m0", "g", "m1", "g",
                     "m0", "d", "m1", "g", "m0", "g", "m1", "d",
                     "m0", "g", "m1", "d", "m0", "g", "m1", "g") + (
                     "m0", "d", "m1", "d", "d", "d", "d", "d")

            def accumulate(half_ap):
                nonlocal add_rr
                r = ROUTE[min(add_rr, len(ROUTE) - 1)]
                add_rr += 1
                if r == "d":
                    if not acc_state["d"]:
                        nc.vector.tensor_copy(acc_d[:], half_ap)
                        acc_state["d"] = True
                    else:
                        nc.vector.tensor_add(acc_d[:], acc_d[:], half_ap)
                elif r == "g":
                    if not acc_state["g"]:
                        # first GpSimd accumulate folds in exponent_B
                        nc.gpsimd.tensor_add(acc_g[:], ebt[:], half_ap)
                        acc_state["g"] = True
                    else:
                        nc.gpsimd.tensor_add(acc_g[:], acc_g[:], half_ap)
                else:
                    am = acc_m[0 if r == "m0" else 1]
                    nc.gpsimd.dma_start(out=am[:], in_=half_ap,
                                        accum_op=mybir.AluOpType.add)

            prodbuf = [None, None]
            for sg in range(NSG):
                ptt = ppool.tile([KDIM, 2 * PW], bf16, tag="ptt")
                nc.sync.dma_start(out=ptt[:], in_=PTT[sg])
                pbase = 0
                pss = []
                for kk in range(2):
                    # one [128, 1024] = [ih0 | ih1] PSUM tensor per k:
                    # 2 banks, bufs=4 lets the PE run two sgs ahead of
                    # the drain engines.
                    ps = pspool.tile([128, 2 * JL], f32, tag="ps", name="ps")
                    pss.append(ps)
                    base = pbase + kk * PW
                    for ih in range(2):
                        nc.tensor.matmul(
                            ps[:, ih * JL:(ih + 1) * JL],
                            lhsT=ptt[:, base + ih * 128:base + (ih + 1) * 128],
                            rhs=ptt[:, base + B:base + PW],
                            start=True,
                            stop=True,
                        )
                if sg in A_SET:
                    for kk in range(2):
                        lg = lgpool.tile([128, 2 * JL], f32, tag="lg",
                                         name="lg")
                        nc.scalar.activation(lg[:], pss[kk][:], AF.Ln)
                        accumulate(lg[:])
                else:
                    ci = b_chunk[sg]
                    for kk in range(2):
                        sl = slice(kk * 2 * JL, (kk + 1) * 2 * JL)
                        if b_pos[sg] == 0:
                            if kk == 0:
                                prodbuf[ci] = pbpool.tile(
                                    [128, 4 * JL], f32,
                                    tag=f"pb{ci}", name=f"pb{ci}")
                            nc.scalar.copy(prodbuf[ci][:, sl], pss[kk][:])
                        else:
                            nc.vector.tensor_mul(
                                prodbuf[ci][:, sl], pss[kk][:],
                                prodbuf[ci][:, sl])
                    if b_last[sg]:
                        clg = clgpool.tile([128, 4 * JL], f32, tag="clg",
                                           name="clg")
                        nc.scalar.activation(clg[:], prodbuf[ci][:], AF.Ln)
                        for h in range(2):
                            accumulate(clg[:, h * 2 * JL:(h + 1) * 2 * JL])

            # merge the accumulator chains; acc_d (which received the
            # final adds) merges last so the tail stays short
            nc.vector.tensor_add(acc_m[0][:], acc_m[0][:], acc_m[1][:])
            nc.vector.tensor_add(acc_g[:], acc_g[:], acc_m[0][:])
            acc = acc_g
            nc.vector.tensor_add(acc[:], acc[:], acc_d[:])

            # Softmax over l, fully batched. Instead of a max-reduce, shift
            # by the l=0 column (per-row log-spread < 14, exp stays in f32
            # range; softmax is shift-invariant). Broadcast APs (stride-0
            # inner dim) let one op handle all 16 (ih,j) groups.
            NG = 2 * JLOC  # 16 groups
            acc3 = acc[:, :].rearrange("p (g l) -> p g l", g=NG)
            c_b = acc[:, ::QU].broadcast_to((128, NG, QU))
            sub = opool.tile([128, 2 * JL], f32, tag="sub")
            sub3 = sub[:, :].rearrange("p (g l) -> p g l", g=NG)
            nc.vector.tensor_sub(sub3, acc3, c_b)
            exs = opool.tile([128, 2 * JL], f32, tag="exs")
            nc.scalar.activation(exs[:], sub[:], AF.Exp)
            exs3 = exs[:, :].rearrange("p (g l) -> p g l", g=NG)
            smb = smpool.tile([128, NG], f32, tag="smb")
            nc.vector.tensor_reduce(
                smb[:], exs3, axis=mybir.AxisListType.X,
                op=mybir.AluOpType.add)
            rcb = smpool.tile([128, NG], f32, tag="rcb")
            nc.vector.reciprocal(rcb[:], smb[:])
            ot = opool.tile([128, 2 * JL], f32, tag="otb", name="otb")
            ot3 = ot[:, :].rearrange("p (g l) -> p g l", g=NG)
            nc.vector.tensor_mul(
                ot3, exs3, rcb[:, :].broadcast_to((128, NG, QU)))
            for ih in range(2):
                nc.sync.dma_start(out=OUT[ih, :, :],
                                  in_=ot[:, ih * JL:(ih + 1) * JL])
    nc.compile()
    return nc


def _host_prep(P, weight, bias_abs, bias_q, lambda_abs, lambda_q):
    """Build per-core input maps. Host-side, cheap (T is ~64MB total)."""
    import ml_dtypes

    bf16 = ml_dtypes.bfloat16
    s1 = (np.arange(QU, dtype=np.float64) / QU)
    s0 = (np.arange(QL, dtype=np.float64) / QL)
    diff2 = (s0[None, :] - s1[:, None]) ** 2             # [l, m]
    # t[j, k, l, m] = T - 1, in bf16 (error scales with |t| <= 0.11)
    t_full = np.expm1(-weight[:, :, None, None].astype(np.float64)
                      * diff2[None, None, :, :]).astype(np.float32)
    sq = s1
    expB = (-bias_q.astype(np.float64) * (sq[None, :] - lambda_q) ** 2
            - bias_abs.astype(np.float64)
            * np.abs(sq[None, :] - lambda_abs)).astype(np.float32)

    P32 = P.astype(np.float32)
    S = P32.sum(axis=2, dtype=np.float64).astype(np.float32)   # [i, k]
    S_hi = S.astype(bf16)
    S_lo = (S - S_hi.astype(np.float32)).astype(bf16)

    PT_bf = P32.transpose(1, 2, 0).astype(bf16)          # [k, m, i]

    in_maps = []
    for c in range(NCORES):
        tc_ = t_full[c * JLOC:(c + 1) * JLOC]            # [8, k, l, m]
        tc_ = tc_.transpose(1, 3, 0, 2).reshape(NL, QL, JL)  # [k, m, (j,l)]
        PTTc = np.empty((NL, KDIM, PW), dtype=bf16)
        PTTc[:, :QL, :B] = PT_bf
        PTTc[:, QL, :B] = S_hi.T                         # row 64: S_hi
        PTTc[:, QL + 1, :B] = S_lo.T                     # row 65: S_lo
        PTTc[:, :QL, B:] = tc_.astype(bf16)
        PTTc[:, QL:, B:] = bf16(1.0)                     # ones against S rows
        # pack two k's per DMA row-block: [32, 66, 1536]
        PTTc = np.ascontiguousarray(
            PTTc.reshape(NL // 2, 2, KDIM, PW).transpose(0, 2, 1, 3)
            .reshape(NL // 2, KDIM, 2 * PW))
        eb_row = np.tile(expB[c * JLOC:(c + 1) * JLOC].reshape(JL), 2)
        EBc = np.ascontiguousarray(np.broadcast_to(eb_row, (128, 2 * JL)))
        in_maps.append({"PTT": PTTc, "EB": EBc})
    return in_maps


_PROGRAM = None


def _get_program():
    global _PROGRAM
    if _PROGRAM is None:
        _PROGRAM = _build_program()
    return _PROGRAM


def run_on_device(in_maps, trace=False):
    from concourse.bass_utils import run_bass_kernel_spmd
    nc = _get_program()
    return run_bass_kernel_spmd(
        nc, in_maps, core_ids=list(range(NCORES)), trace=trace,
    )


def assemble(results):
    out = np.empty((B, NU, QU), dtype=np.float32)
    for c in range(NCORES):
        rc = results[c]["out"].reshape(B, JLOC, QU)
        out[:, c * JLOC:(c + 1) * JLOC, :] = rc
    return out


def kernel(P, weight, bias_abs, bias_q, lambda_abs, lambda_q):
    in_maps = _host_prep(P, weight, bias_abs, bias_q, lambda_abs, lambda_q)
    res = run_on_device(in_maps, trace=False)
    return assemble(res.results)

